# revision 24
# baseline (speedup 1.0000x reference)
"""EntityBoundaryPredictor Bass kernel for 8 trn2 NeuronCores.

Reference computation (B=4, E=16, T=1024, H=1024, fp32):
    t   = token_embedding @ Wt + bt                       # [B,T,H]
    e   = entity_embedding @ We + be                      # [B,E,H]
    cls = einsum('beth,h->bet', relu(t[:,None]+e[:,:,None]), Wp) + bp
    cls = where(token_mask, cls, -1e4); p = sigmoid(cls)  # returns (cls, p)

Sharding: data-parallel over (b, token-half): core s -> b = s//2,
tokens [th*512,(th+1)*512) with th = s%2.  Weights replicated.

Per-core device plan (h kept on SBUF partitions throughout):
  - DMA: inputs split over the three HWDGE rings (scalar/sync/gpsimd) with
    the small consts first; entity embeddings ride inside `smalls` (a
    separate strided transfer would be 1024 tiny packets).
  - PE: warmup dummy matmuls during the DMA head flip the HAM clock gate
    to 8/8 before real work; t'(k,t) = Wt^T @ tokT accumulated over 8
    h-chunks into PSUM (same for e'); ACT folds biases in during the
    PSUM->SBUF copy.
  - DVE/ACT/GpSimd: m = relu(t' + e'_scalar) as fused per-partition-scalar
    ops per (e, h-chunk) [128,512] tile, split across the three engines.
  - PE: cls partial = Wp^T @ m -- an M=32 matvec per (e, h-chunk), packed
    4-wide into PSUM column groups (partitions 0/32/64/96; concurrent in
    the array), accumulators resident in four PSUM banks across the h
    loop.  The mask fill is one extra K=1 accumulate per group:
    += ones[1,128]^T @ ((1-mask)*NEG)[1,T] puts NEG on masked tokens.
  - Finalize: cls = rps + bp (DVE), p = sigmoid(rps + bp) (ACT); DMA the
    4 rows out per entity group, spread across rings.
"""

import os

import numpy as np

import bass_rust as _bass_rust
import concourse.bacc as bacc
import concourse.mybir as mybir
from concourse.hw_specs import get_activation_tables
from concourse.tile import TileContext
from concourse.bass_utils import run_bass_kernel_spmd

B, E, T, H = 4, 16, 1024, 1024
P = 128
NCORES = 8
TS = T // 2          # tokens per core
HC = H // P          # h-chunks (contraction)
KC = H // P          # k-chunks (projected feature dim; == h of stage 2)
NEG = -10000.0

F32 = mybir.dt.float32
F32R = mybir.dt.float32r
BF16 = mybir.dt.bfloat16
F16 = mybir.dt.float16

# smalls layout (uint32 columns): btR[8] beR[8] wpR[8] bpR[1] pad[3] entR[64]
# (pad keeps the ent block 16B-aligned for the PE rhs AP)
NS_F32 = 3 * KC + 1 + 3          # f32 const columns incl pad
NS_ENT = HC * E // 2             # ent f16 data as u32 columns
NS = NS_F32 + NS_ENT

WSCALE = 32.0  # fp8 weight pre-scale (undone by the eviction `scale`)

CFG = {
    "in_dt": os.environ.get("K_IN_DT", "f16"),
    # weight (lhsT) dtype: f8e3 halves the weight DMA bytes; values are
    # host-scaled by WSCALE so N(0,1/H) entries stay in e3m4's normal range
    "w_dt": os.environ.get("K_W_DT", "f8e3"),
    "tp_dt": os.environ.get("K_TP_DT", "f16"),
    "m_dt": os.environ.get("K_M_DT", "f16"),
    "reps": int(os.environ.get("K_REPS", "1")),
    # stage bisection for benchmarking: dma | proj | elem | full
    "stage": os.environ.get("K_STAGE", "full"),
    # relu-tile engine split out of a cycle of 20: gp, act, rest on DVE
    # (gp is pathological: stock Q7 ucode has no fast f16 add+max path and
    # its SBUF-port contention also drags DVE down ~13x)
    "gp_n": int(os.environ.get("K_GP_N", "0")),
    "act_n": int(os.environ.get("K_ACT_N", "5")),
    # warmup dummy matmuls (FD=128) to trip the HAM clock gate early
    "warm": int(os.environ.get("K_WARM", "28")),
    # bisection: include the K=1 mask-fill matmul
    "maskmm": int(os.environ.get("K_MASKMM", "1")),
}

_DT = {"f32": F32, "f32r": F32R, "bf16": BF16, "f16": F16,
       "f8e3": mybir.dt.float8e3}

LAST_RESULTS = None  # BassKernelResults of the most recent run (for test.py)
_BUILT = None        # (cfg_key, nc)


def build(cfg=None):
    cfg = cfg or CFG
    in_dt = _DT[cfg["in_dt"]]
    tp_dt = _DT[cfg["tp_dt"]]
    m_dt = _DT[cfg["m_dt"]]

    w_dt = _DT[cfg["w_dt"]]
    w_scale = 1.0 / WSCALE if cfg["w_dt"] == "f8e3" else 1.0

    nc = bacc.Bacc("TRN2", target_bir_lowering=False, debug=False)

    # All ACT funcs used here (Identity/Relu/Sigmoid) exist in the
    # sigmoid_and_others set; blank the other sets (ids preserved) so one
    # table load suffices.
    def _one_table_set():
        if not any(
            isinstance(i, mybir.InstActivation)
            for b in nc.main_func.blocks
            for i in b.instructions
        ):
            return
        tables = [
            (n, (f if n == "sigmoid_and_others" else set()))
            for n, f in get_activation_tables(nc.m.arch).items()
        ]
        _bass_rust.insert_act_table_loads(nc, tables)

    nc.insert_act_table_loads = _one_table_set

    tokT = nc.declare_dram_parameter("tokT", [H, TS], in_dt, isOutput=False)
    wt = nc.declare_dram_parameter("wt", [H, H], w_dt, isOutput=False)
    we = nc.declare_dram_parameter("we", [H, H], w_dt, isOutput=False)
    smalls = nc.declare_dram_parameter(
        "smalls", [P, NS], mybir.dt.uint32, isOutput=False
    )
    # pre-broadcast mask rows: every partition = (1-mask)*NEG/128
    maskb = nc.declare_dram_parameter("maskb", [P, TS], F16, isOutput=False)

    cls_out = nc.declare_dram_parameter("cls_out", [E, TS], F32, isOutput=True)
    p_out = nc.declare_dram_parameter("p_out", [E, TS], F32, isOutput=True)

    Act = mybir.ActivationFunctionType
    Alu = mybir.AluOpType

    stage = cfg["stage"]
    CYC = 20
    gp_n = cfg["gp_n"]
    act_n = cfg["act_n"]

    with TileContext(nc) as tc:
        with (
            tc.tile_pool(name="const", bufs=1) as cpool,
            tc.tile_pool(name="mt", bufs=16) as mpool,
            tc.tile_pool(name="fin", bufs=2) as fpool,
            tc.tile_pool(name="psA", bufs=2, space="PSUM") as psA,
            tc.tile_pool(name="psB", bufs=1, space="PSUM") as psB,
            tc.tile_pool(name="psR", bufs=1, space="PSUM") as psR,
            tc.tile_pool(name="psW", bufs=1, space="PSUM") as psW,
        ):
            rep_ctx = tc.For_i(0, cfg["reps"], 1) if cfg["reps"] > 1 else None
            if rep_ctx is not None:
                rep_ctx.__enter__()

            # ---- PE warmup: dummy matmuls on a memset scratch tile ----------
            nwarm = cfg["warm"]
            if nwarm > 0:
                scr = cpool.tile([P, P], in_dt, tag="scr")
                nc.gpsimd.memset(scr[:, :], 0.0)
                wps = psW.tile([P, P], F32, tag="ps_warm")
                for _ in range(nwarm):
                    nc.tensor.matmul(
                        wps[0:32, :], lhsT=scr[:, 0:32], rhs=scr[:, :],
                        start=True, stop=True,
                    )

            # ---- input DMAs over the three rings ----------------------------
            # ring1 = scalar (fast in baseline), ring2 = sync, ring3 = gpsimd
            smalls_sb = cpool.tile([P, NS], mybir.dt.uint32, tag="smalls")
            nc.sync.dma_start(out=smalls_sb[:, :], in_=smalls[:, :])
            maskb_sb = cpool.tile([P, TS], F16, tag="maskb")
            nc.sync.dma_start(out=maskb_sb[:, :], in_=maskb[:, :])

            tok_sb = cpool.tile([P, HC, TS], in_dt, tag="tok")
            HHC = HC // 2
            for tih, eng in ((0, nc.scalar), (1, nc.sync)):
                hsl = slice(tih * HHC, (tih + 1) * HHC)
                eng.dma_start(
                    out=tok_sb[:, hsl, :],
                    in_=tokT[tih * (H // 2) : (tih + 1) * (H // 2), :].rearrange(
                        "(hc p) t -> p hc t", p=P
                    ),
                )
            wt_sb = cpool.tile([P, HC, H], w_dt, tag="wt")
            we_sb = cpool.tile([P, HC, H], w_dt, tag="we")
            # wt in 4 k-chunks alternating the two HW rings (earliest kc
            # first); we rides the otherwise-idle gpsimd SWDGE ring
            for ci in range(4):
                eng = nc.scalar if ci % 2 == 0 else nc.sync
                ksl = slice(ci * (H // 4), (ci + 1) * (H // 4))
                eng.dma_start(
                    out=wt_sb[:, :, ksl],
                    in_=wt[:, ksl].rearrange("(hc p) k -> p hc k", p=P),
                )
            for ci in range(2):
                ksl = slice(ci * (H // 2), (ci + 1) * (H // 2))
                nc.gpsimd.dma_start(
                    out=we_sb[:, :, ksl],
                    in_=we[:, ksl].rearrange("(hc p) k -> p hc k", p=P),
                )

            smalls_f32 = smalls_sb[:, 0:NS_F32].bitcast(F32)
            btR_sb = smalls_f32[:, 0:KC]
            beR_sb = smalls_f32[:, KC : 2 * KC]
            wpR_sb = smalls_f32[:, 2 * KC : 3 * KC]
            bpR_sb = smalls_f32[:, 3 * KC : 3 * KC + 1]
            ent_sb = smalls_sb[:, NS_F32:NS].bitcast(F16).rearrange(
                "p (hc e) -> p hc e", e=E
            )

            # ones lhsT for the mask-fill matmul (sums the 128 mask copies)
            ones_sb = cpool.tile([P, P], F16, tag="ones")
            nc.gpsimd.memset(ones_sb[:, :], 1.0)

            # Wp in the reduce-matmul dtype, replicated to 32 lhsT columns
            wp_sb = cpool.tile([P, KC, 32], m_dt, tag="wp")
            for kc in range(KC):
                nc.vector.tensor_copy(
                    out=wp_sb[:, kc, :],
                    in_=wpR_sb[:, kc : kc + 1].broadcast_to([P, 32]),
                )

            # ---- projections ------------------------------------------------
            tp_sb = cpool.tile([P, KC, TS], tp_dt, tag="tp")   # t' [k, t]
            ep_sb = cpool.tile([P, KC, E], F32, tag="ep")      # e' [k, e]
            if stage != "dma":
                for kc in range(KC):
                    ps = psA.tile([P, TS], F32, tag="ps_proj")
                    for hc in range(HC):
                        nc.tensor.matmul(
                            ps[:, :],
                            lhsT=wt_sb[:, hc, kc * P : (kc + 1) * P],
                            rhs=tok_sb[:, hc, :],
                            start=(hc == 0),
                            stop=(hc == HC - 1),
                        )
                    nc.scalar.activation(
                        tp_sb[:, kc, :], ps[:, :], Act.Identity,
                        bias=btR_sb[:, kc : kc + 1], scale=w_scale,
                    )
                    eps = psB.tile([P, E], F32, tag="ps_eproj")
                    for hc in range(HC):
                        nc.tensor.matmul(
                            eps[:, :],
                            lhsT=we_sb[:, hc, kc * P : (kc + 1) * P],
                            rhs=ent_sb[:, hc, :],
                            start=(hc == 0),
                            stop=(hc == HC - 1),
                        )
                    nc.scalar.activation(
                        ep_sb[:, kc, :], eps[:, :], Act.Identity,
                        bias=beR_sb[:, kc : kc + 1], scale=w_scale,
                    )

            # ---- relu(t'+e') + weighted reduction over h (h-outer) ---------
            if stage in ("elem", "full"):
                rps = [psR.tile([P, TS], F32, tag=f"ps_red{eg}",
                                name=f"rps{eg}")
                       for eg in range(E // 4)]
                g_tile = 0
                for hc in range(HC):
                    for e in range(E):
                        eg, j = divmod(e, 4)
                        m = mpool.tile([P, TS], m_dt, tag="m")
                        lane = g_tile % CYC
                        g_tile += 1
                        if lane < gp_n:
                            nc.gpsimd.tensor_scalar(
                                out=m[:, :],
                                in0=tp_sb[:, hc, :],
                                scalar1=ep_sb[:, hc, e : e + 1],
                                scalar2=0.0,
                                op0=Alu.add,
                                op1=Alu.max,
                            )
                        elif lane < gp_n + act_n:
                            nc.scalar.activation(
                                m[:, :], tp_sb[:, hc, :], Act.Relu,
                                bias=ep_sb[:, hc, e : e + 1],
                            )
                        else:
                            nc.vector.tensor_scalar(
                                out=m[:, :],
                                in0=tp_sb[:, hc, :],
                                scalar1=ep_sb[:, hc, e : e + 1],
                                scalar2=0.0,
                                op0=Alu.add,
                                op1=Alu.max,
                            )
                        if stage == "full":
                            nc.tensor.matmul(
                                rps[eg][32 * j : 32 * j + 32, :],
                                lhsT=wp_sb[:, hc, :],
                                rhs=m[:, :],
                                start=(hc == 0),
                                stop=False,
                                tile_position=(0, 32 * j),
                                # the 4 column groups interleave accumulation
                                # in one bank on disjoint partition ranges;
                                # the group tracker is partition-unaware.
                                skip_group_check=True,
                            )

                # ---- mask fill + finalize: +bp, sigmoid, store -------------
                if stage == "full":
                    out_engs = [nc.sync, nc.scalar, nc.gpsimd]
                    for eg in range(E // 4):
                        if cfg["maskmm"]:
                            nc.tensor.matmul(
                                rps[eg][:, :],
                                lhsT=ones_sb[:, :],
                                rhs=maskb_sb[:, :],
                                start=False,
                                stop=True,
                                skip_group_check=True,
                            )
                        clsT = fpool.tile([P, TS], F32, tag="clsT")
                        nc.vector.tensor_scalar_add(
                            clsT[:, :], rps[eg][:, :], bpR_sb[:, 0:1],
                        )
                        pS = fpool.tile([P, TS], F32, tag="pS")
                        nc.scalar.activation(
                            pS[:, :], rps[eg][:, :], Act.Sigmoid,
                            bias=bpR_sb[:, 0:1],
                        )
                        cls_rows = clsT[:, :].rearrange(
                            "(a b) t -> a b t", b=32)[:, 0, :]
                        p_rows = pS[:, :].rearrange(
                            "(a b) t -> a b t", b=32)[:, 0, :]
                        eng = out_engs[eg % 3]
                        eng.dma_start(
                            out=cls_out[eg * 4 : eg * 4 + 4, :], in_=cls_rows
                        )
                        eng.dma_start(
                            out=p_out[eg * 4 : eg * 4 + 4, :], in_=p_rows
                        )

            if rep_ctx is not None:
                rep_ctx.__exit__(None, None, None)

    nc.compile()
    return nc


def _np_dt(name):
    import ml_dtypes

    return {"f32": np.float32, "f32r": np.float32, "bf16": ml_dtypes.bfloat16,
            "f16": np.float16, "f8e3": ml_dtypes.float8_e3m4}[name]


def shard_inputs(token_embedding, entity_embedding, token_mask, Wt, bt, We, be,
                 Wp, bp, cfg=None):
    cfg = cfg or CFG
    ind = _np_dt(cfg["in_dt"])
    f32 = np.float32

    wnd = _np_dt(cfg["w_dt"])
    wsc = WSCALE if cfg["w_dt"] == "f8e3" else 1.0
    wt_s = np.ascontiguousarray((Wt.astype(f32) * wsc).astype(wnd))
    we_s = np.ascontiguousarray((We.astype(f32) * wsc).astype(wnd))
    btR = np.ascontiguousarray(bt.astype(f32).reshape(KC, P).T)
    beR = np.ascontiguousarray(be.astype(f32).reshape(KC, P).T)
    wpR = np.ascontiguousarray(Wp.astype(f32).reshape(KC, P).T)
    bpR = np.broadcast_to(bp.astype(f32).reshape(1, 1), (P, 1))

    consts = np.concatenate(
        [btR.view(np.uint32), beR.view(np.uint32), wpR.view(np.uint32),
         np.ascontiguousarray(bpR).view(np.uint32),
         np.zeros((P, 3), np.uint32)], axis=1,
    )

    in_maps = []
    for s in range(NCORES):
        b, th = divmod(s, 2)
        tsl = slice(th * TS, (th + 1) * TS)
        tokT = np.ascontiguousarray(
            token_embedding[b, tsl, :].T.astype(ind, copy=False))
        # entity embeddings packed into smalls as [p, hc, e] f16 columns
        entR = np.ascontiguousarray(
            entity_embedding[b].T.astype(np.float16)       # [H, E]
            .reshape(HC, P, E).transpose(1, 0, 2)          # [P, HC, E]
            .reshape(P, HC * E)
        ).view(np.uint32)
        smalls = np.ascontiguousarray(np.concatenate([consts, entR], axis=1))
        mrow = ((1.0 - token_mask[b, tsl].astype(f32)) * (NEG / P)).astype(
            np.float16
        )
        maskb = np.ascontiguousarray(np.broadcast_to(mrow[None, :], (P, TS)))
        in_maps.append({
            "tokT": tokT, "wt": wt_s, "we": we_s, "smalls": smalls,
            "maskb": maskb,
        })
    return in_maps


def kernel(token_embedding, entity_embedding, token_mask, Wt, bt, We, be, Wp, bp):
    global LAST_RESULTS, _BUILT
    cfg_key = tuple(sorted(CFG.items()))
    if _BUILT is None or _BUILT[0] != cfg_key:
        _BUILT = (cfg_key, build(CFG))
    nc = _BUILT[1]

    in_maps = shard_inputs(token_embedding, entity_embedding, token_mask,
                           Wt, bt, We, be, Wp, bp)
    trace = os.environ.get("K_TRACE", "0") == "1"
    res = run_bass_kernel_spmd(nc, in_maps, core_ids=list(range(NCORES)),
                               trace=trace)
    LAST_RESULTS = res

    cls = np.empty((B, E, T), np.float32)
    p = np.empty((B, E, T), np.float32)
    for s in range(NCORES):
        b, th = divmod(s, 2)
        tsl = slice(th * TS, (th + 1) * TS)
        cls[b, :, tsl] = res.results[s]["cls_out"]
        p[b, :, tsl] = res.results[s]["p_out"]
    return cls, p
